# revision 2
# baseline (speedup 1.0000x reference)
"""GCN layer kernel for trn2: bf16 host-precomputed tables, fp8 one-hot
matrices fed directly to the PE, grouped 4-queue DMA gathers, 2-round global
histogram threshold with in-bin interpolation, masked-gather-index phase B."""
import sys
sys.path.insert(0, '/opt/trn_rl_repo')
import numpy as np
import ml_dtypes
from dataclasses import dataclass

import concourse.bass as bass
import concourse.bacc as bacc
import concourse.mybir as mybir
import concourse.tile as tile
from concourse.bass_utils import run_bass_kernel_spmd

F32 = mybir.dt.float32
BF16 = mybir.dt.bfloat16
I16 = mybir.dt.int16
FP8 = mybir.dt.float8e4
AF = mybir.ActivationFunctionType
OP = mybir.AluOpType

NODES_PAD = 50176
LSPLIT = 32767
D = 128
CORES = 8
KCUT = 80000
G = 4      # tiles per gather group
SB = 12    # chunks per phase-A sub-batch (PSUM [128, SB*128] f32 = 3 banks)
QUEUES = 4         # SWDGE queues for gathers (1..4)
SINGLE_PACKET = False


@dataclass
class Cfg:
    nodes: int = NODES_PAD
    lsplit: int = LSPLIT
    cores: int = CORES
    d: int = D
    kcut: int = KCUT
    nlmax: int = 0
    nhmax: int = 0

    @property
    def tpc(self):
        return self.nodes // 128 // self.cores

    @property
    def own(self):
        return self.nodes // self.cores

    @property
    def nct(self):
        return self.nlmax + self.nhmax

    @property
    def nchunk(self):
        return self.tpc * self.nct

    @property
    def slots(self):
        return self.nchunk * 128

    @property
    def zl(self):
        return self.lsplit

    @property
    def zh(self):
        return self.nodes - self.lsplit

    @property
    def hrows(self):
        return self.nodes - self.lsplit + 1

    @property
    def groups(self):
        """list of lists of tile ids per gather group"""
        return [list(range(g, min(g + G, self.tpc)))
                for g in range(0, self.tpc, G)]


def make_cfg(src, dst):
    cfg = Cfg()
    own = cfg.own
    core_of = dst // own
    nl, nh = 1, 1
    for c in range(cfg.cores):
        sel = core_of == c
        s, dd = src[sel], dst[sel]
        t_all = (dd.astype(np.int64) - c * own) // 128
        is_h = s >= cfg.lsplit
        cl = np.bincount(t_all[~is_h], minlength=cfg.tpc)
        ch = np.bincount(t_all[is_h], minlength=cfg.tpc)
        nl = max(nl, int(np.ceil(cl.max() / 128)))
        nh = max(nh, int(np.ceil(ch.max() / 128)))
    cfg.nlmax, cfg.nhmax = nl, nh
    return cfg


def slot_layout(cfg):
    """Group-major slot order: per group, all L chunks (tile-major), then all
    H chunks. Returns (chunk_tile, chunk_is_h, chunk_cof) lists over global
    chunk index, all identical across cores."""
    chunk_tile, chunk_is_h = [], []
    for tiles in cfg.groups:
        for t in tiles:
            chunk_tile += [t] * cfg.nlmax
            chunk_is_h += [0] * cfg.nlmax
        for t in tiles:
            chunk_tile += [t] * cfg.nhmax
            chunk_is_h += [1] * cfg.nhmax
    return chunk_tile, chunk_is_h


def host_prep(cfg: Cfg, features, W, src, dst):
    n_real = features.shape[0]
    deg = np.bincount(dst, minlength=cfg.nodes).astype(np.float32)
    norm = np.clip(deg, 1.0, None) ** -0.5
    featp = np.zeros((cfg.nodes, cfg.d), np.float32)
    featp[:n_real] = features
    l2 = np.linalg.norm(featp, axis=1, keepdims=True)
    nh = featp / np.clip(l2, 1e-12, None)
    ft = featp * norm[:, None]

    def split_tab(a):
        L = np.zeros((cfg.lsplit + 1, cfg.d), np.float32)
        H = np.zeros((cfg.hrows, cfg.d), np.float32)
        L[:cfg.lsplit] = a[:cfg.lsplit]
        H[:cfg.nodes - cfg.lsplit] = a[cfg.lsplit:]
        return (L.astype(ml_dtypes.bfloat16), H.astype(ml_dtypes.bfloat16))

    nhL, nhH = split_tab(nh)
    ftL, ftH = split_tab(ft)
    wT = np.ascontiguousarray(W.T).astype(ml_dtypes.bfloat16)
    # selmat[qm][j, p] = (j == 16*qm + p%16)
    j = np.arange(128).reshape(128, 1)
    p = np.arange(128).reshape(1, 128)
    selmat = np.concatenate(
        [(j == 16 * qm + p % 16) for qm in range(8)],
        axis=1).astype(ml_dtypes.float8_e4m3)  # [128, 8*128]

    chunk_tile, chunk_is_h = slot_layout(cfg)
    chunk_tile = np.array(chunk_tile)
    chunk_is_h = np.array(chunk_is_h)

    core_of = dst // cfg.own
    in_maps = []
    for c in range(cfg.cores):
        sel = np.nonzero(core_of == c)[0]
        s = src[sel].astype(np.int64)
        dloc_all = dst[sel].astype(np.int64) - c * cfg.own
        t_all = dloc_all // 128
        loc_all = dloc_all % 128
        is_h = (s >= cfg.lsplit).astype(np.int64)
        gidx = np.zeros(cfg.slots, np.int64)
        zvec = np.zeros(cfg.slots, np.int64)
        dstloc = np.full(cfg.slots, -1, np.int64)
        # chunk ranges per (tile, is_h) in the group-major order
        cstart = {}
        pos = 0
        for cc in range(cfg.nchunk):
            key = (chunk_tile[cc], chunk_is_h[cc])
            if key not in cstart:
                cstart[key] = cc
        for t in range(cfg.tpc):
            for hs in (0, 1):
                mm = (t_all == t) & (is_h == hs)
                n = int(mm.sum())
                budget = (cfg.nhmax if hs else cfg.nlmax) * 128
                assert n <= budget
                zval = cfg.zh if hs else cfg.zl
                c0 = cstart[(t, hs)] * 128
                sl = slice(c0, c0 + budget)
                gi = np.full(budget, zval, np.int64)
                gi[:n] = s[mm] - (cfg.lsplit if hs else 0)
                gidx[sl] = gi
                zvec[sl] = zval
                dl = np.full(budget, -1, np.int64)
                dl[:n] = loc_all[mm]
                dstloc[sl] = dl
        gdelta = gidx - zvec

        def wrap16(a):
            return np.ascontiguousarray(
                np.tile(a.reshape(-1, 16).T, (8, 1)).astype(np.int16))

        gidx_w = wrap16(gidx)
        gdelta_w = wrap16(gdelta)
        zvec_w = wrap16(zvec)
        sexp = (dstloc.reshape(1, -1) ==
                np.arange(128).reshape(128, 1)).astype(ml_dtypes.float8_e4m3)
        dl2 = dstloc.reshape(cfg.nchunk, 128)
        sexpT = (dl2.T[:, :, None] == np.arange(128)[None, None, :])
        sexpT = np.ascontiguousarray(
            sexpT.reshape(128, cfg.nchunk * 128).astype(ml_dtypes.float8_e4m3))

        base = c * cfg.own
        own_slice = slice(base, base + cfg.own)
        nhiw = np.ascontiguousarray(
            nh[own_slice].reshape(cfg.tpc, 128, cfg.d).transpose(1, 0, 2)
            .reshape(128, cfg.tpc * cfg.d)).astype(ml_dtypes.bfloat16)
        hpw = np.ascontiguousarray(
            featp[own_slice].reshape(cfg.tpc, 128, cfg.d).transpose(1, 0, 2)
            .reshape(128, cfg.tpc * cfg.d)).astype(np.float32)
        normw = np.ascontiguousarray(
            norm[own_slice].reshape(cfg.tpc, 128).T).astype(np.float32)
        in_maps.append(dict(
            nhL=nhL, nhH=nhH, ftL=ftL, ftH=ftH, wT=wT, selmat=selmat,
            gidx=gidx_w, gdelta=gdelta_w, zvec=zvec_w,
            sexp=sexp, sexpT=sexpT,
            nhiw=nhiw, hpw=hpw, normw=normw,
            iota32=np.arange(32, dtype=np.float32).reshape(1, 32),
            onescol=np.ones((128, 1), np.float32),
            onesrow=np.ones((1, 128), np.float32),
        ))
    return in_maps


def build_nc(cfg: Cfg):
    nc = bacc.Bacc(None)
    d = cfg.d
    TPC, NCT, NCH = cfg.tpc, cfg.nct, cfg.nchunk
    NL, NH = cfg.nlmax, cfg.nhmax
    SL16 = cfg.slots // 16
    KC = float(cfg.kcut)
    W1 = 2.0 / 32
    W2 = 2.0 / 1024
    chunk_tile, chunk_is_h = slot_layout(cfg)

    nhL = nc.dram_tensor("nhL", [cfg.lsplit + 1, d], BF16, kind="ExternalInput")
    nhH = nc.dram_tensor("nhH", [cfg.hrows, d], BF16, kind="ExternalInput")
    ftL = nc.dram_tensor("ftL", [cfg.lsplit + 1, d], BF16, kind="ExternalInput")
    ftH = nc.dram_tensor("ftH", [cfg.hrows, d], BF16, kind="ExternalInput")
    wT_e = nc.dram_tensor("wT", [d, d], BF16, kind="ExternalInput")
    selmat_e = nc.dram_tensor("selmat", [128, 8 * 128], FP8,
                              kind="ExternalInput")
    gidx_e = nc.dram_tensor("gidx", [128, SL16], I16, kind="ExternalInput")
    gdelta_e = nc.dram_tensor("gdelta", [128, SL16], I16, kind="ExternalInput")
    zvec_e = nc.dram_tensor("zvec", [128, SL16], I16, kind="ExternalInput")
    sexp_e = nc.dram_tensor("sexp", [128, cfg.slots], FP8, kind="ExternalInput")
    sexpT_e = nc.dram_tensor("sexpT", [128, cfg.slots], FP8,
                             kind="ExternalInput")
    nhiw_e = nc.dram_tensor("nhiw", [128, TPC * d], BF16, kind="ExternalInput")
    hpw_e = nc.dram_tensor("hpw", [128, TPC * d], F32, kind="ExternalInput")
    normw_e = nc.dram_tensor("normw", [128, TPC], F32, kind="ExternalInput")
    iota32_e = nc.dram_tensor("iota32", [1, 32], F32, kind="ExternalInput")
    onescol_e = nc.dram_tensor("onescol", [128, 1], F32, kind="ExternalInput")
    onesrow_e = nc.dram_tensor("onesrow", [1, 128], F32, kind="ExternalInput")
    h_ext = nc.dram_tensor("h", [cfg.own, d], F32, kind="ExternalOutput")

    cc_in = nc.dram_tensor("cc_in", [1, 32], F32)
    cc_out = nc.dram_tensor("cc_out", [1, 32], F32, addr_space="Shared")
    groups_all = [list(range(cfg.cores))]

    with tile.TileContext(nc) as tc:
        with (tc.tile_pool(name="const", bufs=1) as cpool,
              tc.tile_pool(name="state", bufs=1) as spool,
              tc.tile_pool(name="gath", bufs=2) as gpool,
              tc.tile_pool(name="sest", bufs=2) as sepool,
              tc.tile_pool(name="work", bufs=2) as fpool,
              tc.tile_pool(name="hps", bufs=2) as hpool,
              tc.tile_pool(name="ypsum", bufs=2, space="PSUM") as ypool,
              tc.tile_pool(name="absum", bufs=2, space="PSUM") as abpool,
              tc.tile_pool(name="misc", bufs=2) as mpool,
              tc.tile_pool(name="thr", bufs=1) as tpool):

            # resident loads
            gidx_sb = spool.tile([128, SL16], I16, tag="gidx")
            nc.sync.dma_start(gidx_sb[:], gidx_e[:])
            nhiw = spool.tile([128, TPC * d], BF16, tag="nhiw")
            nc.sync.dma_start(nhiw[:], nhiw_e[:])
            wT_sb = cpool.tile([d, d], BF16, tag="wT")
            nc.sync.dma_start(wT_sb[:], wT_e[:])
            selmat = cpool.tile([128, 8 * 128], FP8, tag="selmat")
            nc.sync.dma_start(selmat[:], selmat_e[:])
            normw = spool.tile([128, TPC], F32, tag="normw")
            nc.sync.dma_start(normw[:], normw_e[:])
            gdelta_sb = spool.tile([128, SL16], I16, tag="gdelta")
            nc.sync.dma_start(gdelta_sb[:], gdelta_e[:])
            zvec_sb = spool.tile([128, SL16], I16, tag="zvec")
            nc.sync.dma_start(zvec_sb[:], zvec_e[:])
            iota32 = cpool.tile([1, 32], F32, tag="iota32")
            nc.sync.dma_start(iota32[:], iota32_e[:])
            ones_col = cpool.tile([128, 1], F32, tag="onescol")
            nc.sync.dma_start(ones_col[:], onescol_e[:])
            ones_row = cpool.tile([1, 128], F32, tag="onesrow")
            nc.sync.dma_start(ones_row[:], onesrow_e[:])

            cos_sb = spool.tile([128, NCH], F32, tag="cos")

            # ---------- Phase A: cos ----------
            gc0 = 0  # global chunk cursor
            for gi, tiles in enumerate(cfg.groups):
                ng = len(tiles)
                gchunks = ng * NCT  # chunks in this group
                i0 = gc0 * 8       # gidx column offset
                x_t = gpool.tile([128, G * NCT, d], BF16, tag="x")
                nc.gpsimd.dma_gather(
                    out_ap=x_t[:, :ng * NL, :], in_ap=nhL[:],
                    idxs_ap=gidx_sb[:, i0:i0 + ng * NL * 8],
                    num_idxs=ng * NL * 128, num_idxs_reg=ng * NL * 128,
                    elem_size=d, single_packet=False)
                nc.gpsimd.dma_gather(
                    out_ap=x_t[:, ng * NL:ng * NCT, :], in_ap=nhH[:],
                    idxs_ap=gidx_sb[:, i0 + ng * NL * 8:i0 + gchunks * 8],
                    num_idxs=ng * NH * 128, num_idxs_reg=ng * NH * 128,
                    elem_size=d, single_packet=False)
                se = sepool.tile([128, G * NCT * 128], FP8, tag="se")
                nc.sync.dma_start(
                    se[:, :gchunks * 128],
                    sexp_e[:, gc0 * 128:(gc0 + gchunks) * 128])
                # sub-batches of SB chunks
                for s0 in range(0, gchunks, SB):
                    sn = min(SB, gchunks - s0)
                    y_ps = ypool.tile([128, SB * 128], F32, tag="y")
                    for ci in range(sn):
                        gc = s0 + ci
                        t = chunk_tile[gc0 + gc]
                        nc.tensor.matmul(
                            y_ps[:, ci * 128:(ci + 1) * 128],
                            se[:, gc * 128:(gc + 1) * 128],
                            nhiw[:, t * d:(t + 1) * d],
                            start=True, stop=True)
                    y16 = fpool.tile([128, SB * 128], BF16, tag="y16")
                    nc.scalar.copy(y16[:, :sn * 128], y_ps[:, :sn * 128])
                    prod = fpool.tile([128, SB * 128], BF16, tag="prod")
                    nc.vector.tensor_mul(
                        prod[:, :sn * 128],
                        x_t[:, s0:s0 + sn, :].rearrange("p c x -> p (c x)"),
                        y16[:, :sn * 128])
                    nc.vector.tensor_reduce(
                        cos_sb[:, gc0 + s0:gc0 + s0 + sn],
                        prod[:, :sn * 128].rearrange("p (c x) -> p c x",
                                                     x=128),
                        axis=mybir.AxisListType.X, op=OP.add)
                gc0 += gchunks

            # prefetch first phase-B seT while threshold runs
            seT0 = sepool.tile([128, G * NCT * 128], FP8, tag="seT")
            nc.sync.dma_start(seT0[:, :G * NCT * 128],
                              sexpT_e[:, :G * NCT * 128])

            # ---------- Phase T: threshold ----------
            cos16 = tpool.tile([128, NCH], BF16, tag="cos16")
            nc.scalar.copy(cos16[:], cos_sb[:])
            lo_t = tpool.tile([1, 1], F32, tag="lo")
            nc.vector.memset(lo_t[:], -1.0)
            th_row = tpool.tile([1, 32], F32, tag="throw")
            th_bc = tpool.tile([128, 32], F32, tag="thbc")
            cnt128 = tpool.tile([128, 32], F32, tag="cnt128")
            gcnt = tpool.tile([1, 32], F32, tag="gcnt")
            srow = tpool.tile([1, 1], F32, tag="srow")
            msct = tpool.tile([1, 32], F32, tag="msct")
            cscr = tpool.tile([128, NCH], BF16, tag="cscr")
            for w_bin in (W1, W2):
                nc.vector.tensor_scalar_mul(th_row[:], iota32[:], w_bin)
                nc.vector.tensor_scalar(th_row[:], th_row[:], lo_t[:], None,
                                        op0=OP.add)
                ps = abpool.tile([128, 128], F32, tag="ab")
                nc.tensor.matmul(ps[:, :32], ones_row[:], th_row[:],
                                 start=True, stop=True)
                nc.vector.tensor_copy(th_bc[:], ps[:, :32])
                for j in range(32):
                    nc.vector.tensor_scalar(
                        cscr[:], cos16[:], th_bc[:, j:j + 1], None,
                        op0=OP.is_lt, op1=OP.add,
                        accum_out=cnt128[:, j:j + 1])
                cps = abpool.tile([128, 128], F32, tag="ab")
                nc.tensor.matmul(cps[:1, :32], ones_col[:], cnt128[:],
                                 start=True, stop=True)
                nc.vector.tensor_copy(gcnt[:], cps[:1, :32])
                nc.sync.dma_start(cc_in[:], gcnt[:])
                nc.gpsimd.collective_compute(
                    "AllReduce", OP.add, replica_groups=groups_all,
                    ins=[cc_in[:]], outs=[cc_out[:]])
                nc.sync.dma_start(gcnt[:], cc_out[:])
                nc.vector.tensor_scalar(msct[:], gcnt[:], KC - 0.5, None,
                                        op0=OP.is_lt)
                nc.vector.tensor_scalar(msct[:], msct[:], 0.0, None,
                                        op0=OP.add, op1=OP.add,
                                        accum_out=srow[:])
                nc.vector.tensor_scalar(srow[:], srow[:], -1.0, 0.0,
                                        op0=OP.add, op1=OP.max)
                nc.vector.scalar_tensor_tensor(
                    lo_t[:], srow[:], w_bin, lo_t[:], op0=OP.mult, op1=OP.add)
            # interpolate t* within the selected bin: t* = lo + W2*clip((k-c_s)/(c_{s+1}-c_s),0,1)
            cs = tpool.tile([1, 1], F32, tag="cs")
            cs1 = tpool.tile([1, 1], F32, tag="cs1")
            s1row = tpool.tile([1, 1], F32, tag="s1row")
            iscr = tpool.tile([1, 32], F32, tag="iscr")
            nc.vector.scalar_tensor_tensor(
                iscr[:], iota32[:], srow[:], gcnt[:],
                op0=OP.is_equal, op1=OP.mult, accum_out=cs[:])
            nc.vector.tensor_scalar_add(s1row[:], srow[:], 1.0)
            nc.vector.scalar_tensor_tensor(
                iscr[:], iota32[:], s1row[:], gcnt[:],
                op0=OP.is_equal, op1=OP.mult, accum_out=cs1[:])
            den = tpool.tile([1, 1], F32, tag="den")
            nc.vector.tensor_sub(den[:], cs1[:], cs[:])
            nc.vector.tensor_scalar_max(den[:], den[:], 0.5)
            nc.vector.reciprocal(den[:], den[:])
            frac = tpool.tile([1, 1], F32, tag="frac")
            nc.vector.tensor_scalar(frac[:], cs[:], -1.0, KC,
                                    op0=OP.mult, op1=OP.add)
            nc.vector.tensor_mul(frac[:], frac[:], den[:])
            nc.vector.tensor_scalar_max(frac[:], frac[:], 0.0)
            nc.vector.tensor_scalar(frac[:], frac[:], 1.0, None, op0=OP.min)
            nc.vector.scalar_tensor_tensor(
                lo_t[:], frac[:], W2, lo_t[:], op0=OP.mult, op1=OP.add)
            pst = abpool.tile([128, 128], F32, tag="ab")
            nc.tensor.matmul(pst[:, :1], ones_row[:], lo_t[:], start=True,
                             stop=True)
            tstar = tpool.tile([128, 1], F32, tag="tstar")
            nc.vector.tensor_copy(tstar[:], pst[:, :1])

            # keep (bf16) -> wrapped via 8 one-hot matmuls -> masked indices
            keep16 = tpool.tile([128, NCH], BF16, tag="keep16")
            nc.vector.tensor_scalar(keep16[:], cos16[:], tstar[:], None,
                                    op0=OP.is_ge)
            gidxB = tpool.tile([128, SL16], I16, tag="gidxB")
            gB_v = gidxB[:].rearrange("p (m e) -> p m e", e=8)
            gd_v = gdelta_sb[:].rearrange("p (m e) -> p m e", e=8)
            for qm in range(8):
                kwps = ypool.tile([128, SB * 128], F32, tag="y")
                for c0 in range(0, NCH, 512):
                    c1 = min(c0 + 512, NCH)
                    nc.tensor.matmul(kwps[:, c0:c1],
                                     selmat[:, qm * 128:(qm + 1) * 128],
                                     keep16[:, c0:c1], start=True, stop=True)
                nc.vector.scalar_tensor_tensor(
                    gB_v[:, :, qm], kwps[:, :NCH], 1.0, gd_v[:, :, qm],
                    op0=OP.mult, op1=OP.mult)
            nc.vector.tensor_add(gidxB[:], gidxB[:], zvec_sb[:])

            # ---------- Phase B: aggregate + linear + tail ----------
            gc0 = 0
            for gi, tiles in enumerate(cfg.groups):
                ng = len(tiles)
                gchunks = ng * NCT
                i0 = gc0 * 8
                xf = gpool.tile([128, G * NCT, d], BF16, tag="x")
                nc.gpsimd.dma_gather(
                    out_ap=xf[:, :ng * NL, :], in_ap=ftL[:],
                    idxs_ap=gidxB[:, i0:i0 + ng * NL * 8],
                    num_idxs=ng * NL * 128, num_idxs_reg=ng * NL * 128,
                    elem_size=d, single_packet=False)
                nc.gpsimd.dma_gather(
                    out_ap=xf[:, ng * NL:ng * NCT, :], in_ap=ftH[:],
                    idxs_ap=gidxB[:, i0 + ng * NL * 8:i0 + gchunks * 8],
                    num_idxs=ng * NH * 128, num_idxs_reg=ng * NH * 128,
                    elem_size=d, single_packet=False)
                if gi == 0:
                    seT = seT0
                else:
                    seT = sepool.tile([128, G * NCT * 128], FP8, tag="seT")
                    nc.sync.dma_start(
                        seT[:, :gchunks * 128],
                        sexpT_e[:, gc0 * 128:(gc0 + gchunks) * 128])
                for ti, t in enumerate(tiles):
                    hp_t = hpool.tile([128, d], F32, tag="hp")
                    nc.sync.dma_start(hp_t[:],
                                      hpw_e[:, t * d:(t + 1) * d])
                    at_ps = abpool.tile([128, 128], F32, tag="ab")
                    tchunks = ([ti * NL + q for q in range(NL)] +
                               [ng * NL + ti * NH + q for q in range(NH)])
                    for k, gc in enumerate(tchunks):
                        nc.tensor.matmul(at_ps[:], xf[:, gc, :],
                                         seT[:, gc * 128:(gc + 1) * 128],
                                         start=(k == 0),
                                         stop=(k == len(tchunks) - 1))
                    at16 = mpool.tile([128, 128], BF16, tag="at16")
                    nc.scalar.copy(at16[:], at_ps[:])
                    h_ps = abpool.tile([128, 128], F32, tag="ab")
                    nc.tensor.matmul(h_ps[:], at16[:], wT_sb[:], start=True,
                                     stop=True)
                    hre = mpool.tile([128, 128], F32, tag="hre")
                    nc.scalar.activation(hre[:], h_ps[:], AF.Relu,
                                         scale=normw[:, t:t + 1])
                    hout = mpool.tile([128, 128], F32, tag="hout")
                    nc.vector.tensor_add(hout[:], hre[:], hp_t[:])
                    nc.sync.dma_start(
                        h_ext[:].rearrange("(t p) x -> p t x", p=128)[:, t, :],
                        hout[:])
                gc0 += gchunks

    nc.finalize()
    return nc


def run(cfg, features, W, src, dst):
    in_maps = host_prep(cfg, features, W, src, dst)
    nc = build_nc(cfg)
    r = run_bass_kernel_spmd(nc, in_maps, core_ids=list(range(cfg.cores)))
    h = np.concatenate([r.results[c]["h"] for c in range(cfg.cores)], axis=0)
    return h[:features.shape[0]]


def kernel(features, W, src, dst):
    src = np.asarray(src).astype(np.int32)
    dst = np.asarray(dst).astype(np.int32)
    features = np.asarray(features, dtype=np.float32)
    W = np.asarray(W, dtype=np.float32)
    cfg = make_cfg(src, dst)
    return run(cfg, features, W, src, dst).astype(np.float32)


if __name__ == "__main__":
    import simrun
    d = np.load('/root/problem/_ref_cache.npz')
    features, W, src, dst = d['features'], d['W'], d['src'], d['dst']
    cfg = make_cfg(src, dst)
    in_maps = host_prep(cfg, features, W, src, dst)
    nc = build_nc(cfg)
    sim = simrun.simulate1(nc, in_maps[0])
